# revision 31
# baseline (speedup 1.0000x reference)
"""EnhancedVectorQuantizer forward on 8 Trainium2 NeuronCores.

Data-parallel over the flattened-token axis (65536 tokens -> 8 x 8192).
Codebook [1024, 256] replicated. Per core the Bass/Tile kernel computes:
  - scores s[t,k] = x_t . e_k - |e_k|^2/2  (fp32 matmul; -e2/2 folded into the
    PSUM accumulation as a bf16x3-split bias row so no extra DVE pass)
  - argmax_k s = argmin_k |x_t - e_k|^2 (DVE max8 + max_index)
  - quantized rows via indirect-DMA gather from the DRAM codebook
  - per-code assignment counts (POOL one-hot, bf16 accumulate on DVE, one
    PE ones-matmul cross-partition reduce at the end)
  - partial sums for the losses:
      commitment: sum|x|^2 - 2*sum s_max   (algebraic identity, exact)
      diversity:  this core's 128-row slab of sum max(|cbn cbn^T|, 0.1)^2
The host only concatenates shards and does the O(K) scalar combine.
"""

import numpy as np
import ml_dtypes

import concourse.bacc as bacc
import concourse.bass as bass
import concourse.mybir as mybir
import concourse.tile as tile
from concourse import bass_utils
from concourse.bass import IndirectOffsetOnAxis

N_CORES = 8
D = 256
K = 1024
P = 128
T_FULL = 8192  # tokens per core

F32 = mybir.dt.float32
BF16 = mybir.dt.bfloat16
U16 = mybir.dt.uint16
U32 = mybir.dt.uint32

A = mybir.AluOpType
AF = mybir.ActivationFunctionType

_NC_CACHE = {}


def _emit_div(nc, sb, cl0, cl1, cr0, cr1, divacc, ps_d):
    # diversity: this core's 128-row slab of sim = cbn @ cbn.T
    for c in range(2):
        cs = slice(c * 512, (c + 1) * 512)
        d_ps = ps_d.tile([P, 512], F32, tag="d_ps")
        nc.tensor.matmul(d_ps[:], cl0[:], cr0[:, cs], start=True, stop=False)
        nc.tensor.matmul(d_ps[:], cl1[:], cr1[:, cs], start=False, stop=True)
        # max(|s|, 0.1)^2 == max(s^2, 0.01)
        divsq = sb.tile([P, 512], F32, tag="divsq")
        nc.scalar.activation(divsq[:], d_ps[:], AF.Square)
        divtmp = sb.tile([P, 512], F32, tag="divtmp")
        nc.vector.tensor_scalar(divtmp[:], divsq[:], 0.01, None, A.max,
                                A.add, accum_out=divacc[:, c:c + 1])


def _emit(nc, tc, io, T):
    NT = T // P           # token tiles
    QB = 4                # tiles per quantized-output DMA slab
    XCH = max(1, NT // 8)  # xt column-chunk size in tiles

    xt, et, er, bias3, cl, cr = (io[k] for k in ("xt", "et", "er", "bias3", "cl", "cr"))
    q, idxo, counts, scal = (io[k] for k in ("q", "idxo", "counts", "scal"))

    import contextlib
    ctx = contextlib.ExitStack()
    with ctx:
        const = ctx.enter_context(tc.tile_pool(name="const", bufs=1))
        sb = ctx.enter_context(tc.tile_pool(name="sb", bufs=3))
        small = ctx.enter_context(tc.tile_pool(name="small", bufs=2))
        qpool = ctx.enter_context(tc.tile_pool(name="qpool", bufs=2))
        ps_s = ctx.enter_context(tc.tile_pool(name="ps_s", bufs=2, space="PSUM"))
        ps_c = ctx.enter_context(tc.tile_pool(name="ps_c", bufs=1, space="PSUM"))
        ps_d = ctx.enter_context(tc.tile_pool(name="ps_d", bufs=1, space="PSUM"))
        ps_x = ctx.enter_context(tc.tile_pool(name="ps_x", bufs=1, space="PSUM"))

        # --- constants / whole-tensor loads ---
        et0 = const.tile([P, K], F32, tag="et0")
        et1 = const.tile([P, K], F32, tag="et1")
        nc.sync.dma_start(et0[:], et[0:P, :])
        nc.sync.dma_start(et1[:], et[P:D, :])

        bias3_sb = const.tile([3, K], BF16, tag="bias3")
        nc.sync.dma_start(bias3_sb[:], bias3[:, :])
        ones3 = const.tile([3, P], BF16, tag="ones3")
        nc.vector.memset(ones3[:], 1.0)
        ones128 = const.tile([P, 1], BF16, tag="ones128")
        nc.vector.memset(ones128[:], 1.0)
        ones1f = const.tile([P, 1], F32, tag="ones1f")
        nc.vector.memset(ones1f[:], 1.0)


        # xt in column chunks so early tiles don't wait on the full 8 MiB
        n_xch = NT // XCH
        xt_chunks = []
        for cc in range(n_xch):
            c0 = const.tile([P, XCH * P], F32, tag=f"xt0_{cc}")
            c1 = const.tile([P, XCH * P], F32, tag=f"xt1_{cc}")
            s0, s1 = cc * XCH * P, (cc + 1) * XCH * P
            if cc == 0:
                # load the first chunk tile-by-tile so tile 0 starts ASAP
                for tt in range(XCH):
                    nc.sync.dma_start(c0[:, tt * P:(tt + 1) * P],
                                      xt[0:P, s0 + tt * P:s0 + (tt + 1) * P])
                    nc.sync.dma_start(c1[:, tt * P:(tt + 1) * P],
                                      xt[P:D, s0 + tt * P:s0 + (tt + 1) * P])
            else:
                nc.sync.dma_start(c0[:], xt[0:P, s0:s1])
                nc.sync.dma_start(c1[:], xt[P:D, s0:s1])
            xt_chunks.append((c0, c1))

        # diversity inputs: load early so the tail doesn't wait on DMA
        cl0 = const.tile([P, P], F32, tag="cl0")
        cl1 = const.tile([P, P], F32, tag="cl1")
        nc.sync.dma_start(cl0[:], cl[0:P, :])
        nc.sync.dma_start(cl1[:], cl[P:D, :])
        cr0 = const.tile([P, K], F32, tag="cr0")
        cr1 = const.tile([P, K], F32, tag="cr1")
        nc.sync.dma_start(cr0[:], cr[0:P, :])
        nc.sync.dma_start(cr1[:], cr[P:D, :])

        # accumulators
        idx_all = const.tile([P, NT], U16, tag="idx_all")
        smax_all = const.tile([P, NT], F32, tag="smax_all")
        xsq_all = const.tile([P, 2 * NT], F32, tag="xsq_all")
        counts_acc = const.tile([P, K], BF16, tag="counts_acc")
        nc.vector.memset(counts_acc[:], 0.0)
        divacc = const.tile([P, 2], F32, tag="divacc")

        # --- main loop over token tiles ---
        for j in range(NT):
            cc, jj = divmod(j, XCH)
            x0 = xt_chunks[cc][0][:, jj * P:(jj + 1) * P]
            x1 = xt_chunks[cc][1][:, jj * P:(jj + 1) * P]

            s_ps = ps_s.tile([P, K], F32, tag="s_ps")
            for c in range(2):
                cs = slice(c * 512, (c + 1) * 512)
                nc.tensor.matmul(s_ps[:, cs], x0, et0[:, cs], start=True, stop=False)
                nc.tensor.matmul(s_ps[:, cs], x1, et1[:, cs], start=False, stop=False)
                nc.tensor.matmul(s_ps[:, cs], ones3[:], bias3_sb[:, cs],
                                 start=False, stop=True)

            s_sb = sb.tile([P, K], F32, tag="s_sb")
            nc.scalar.activation(s_sb[:], s_ps[:], AF.Copy)

            max8 = small.tile([P, 8], F32, tag="max8")
            nc.vector.max(max8[:], s_sb[:])
            idx8 = small.tile([P, 8], U32, tag="idx8")
            nc.vector.max_index(idx8[:], max8[:], s_sb[:])

            # one-hot: score equals the per-token max (f32-exact tie prob ~0)
            oh = sb.tile([P, K], BF16, tag="oh")
            nc.gpsimd.tensor_scalar(oh[:], s_sb[:], max8[:, 0:1], None, A.is_ge)
            nc.vector.tensor_tensor(counts_acc[:], counts_acc[:], oh[:], op=A.add)

            nc.vector.tensor_copy(idx_all[:, j:j + 1], idx8[:, 0:1])
            nc.scalar.activation(smax_all[:, j:j + 1], max8[:, 0:1], AF.Copy)

            dummy = small.tile([P, P], F32, tag="dummy")
            nc.scalar.activation(dummy[:], x0, AF.Square,
                                 accum_out=xsq_all[:, 2 * j:2 * j + 1])
            dummy2 = small.tile([P, P], F32, tag="dummy")
            nc.scalar.activation(dummy2[:], x1, AF.Square,
                                 accum_out=xsq_all[:, 2 * j + 1:2 * j + 2])

            # gather quantized rows for this tile
            a = j % QB
            if a == 0:
                q_sb = qpool.tile([P, QB * D], F32, tag="q_sb")
            nc.gpsimd.indirect_dma_start(
                out=q_sb[:, a * D:(a + 1) * D],
                out_offset=None,
                in_=er[:, :],
                in_offset=IndirectOffsetOnAxis(ap=idx8[:, 0:1], axis=0),
            )
            if a == QB - 1:
                j0 = j - (QB - 1)
                slab = q[j0 * P:(j0 + QB) * P, :].rearrange(
                    "(a p) d -> p a d", p=P)
                nc.sync.dma_start(slab, q_sb[:].rearrange("p (a d) -> p a d", d=D))

        _emit_div(nc, sb, cl0, cl1, cr0, cr1, divacc, ps_d)

        # --- counts: cross-partition reduce of the bf16 accumulator ---
        counts_ps = ps_c.tile([1, K], F32)
        nc.tensor.matmul(counts_ps[:, 0:512], ones128[:], counts_acc[:, 0:512],
                         start=True, stop=True)
        nc.tensor.matmul(counts_ps[:, 512:1024], ones128[:], counts_acc[:, 512:1024],
                         start=True, stop=True)


        # --- final reductions -> scal = [sum s_max, sum |x|^2, div_part, 0] ---
        smax_red = small.tile([P, 1], F32, tag="smax_red")
        nc.vector.reduce_sum(smax_red[:], smax_all[:], axis=mybir.AxisListType.X)
        xsq_red = small.tile([P, 1], F32, tag="xsq_red")
        nc.vector.reduce_sum(xsq_red[:], xsq_all[:], axis=mybir.AxisListType.X)
        div_red = small.tile([P, 1], F32, tag="div_red")
        nc.vector.reduce_sum(div_red[:], divacc[:], axis=mybir.AxisListType.X)

        scal_ps = ps_x.tile([1, 4], F32)
        nc.tensor.matmul(scal_ps[:, 0:1], smax_red[:], ones1f[:], start=True, stop=True)
        nc.tensor.matmul(scal_ps[:, 1:2], xsq_red[:], ones1f[:], start=True, stop=True)
        nc.tensor.matmul(scal_ps[:, 2:3], div_red[:], ones1f[:], start=True, stop=True)

        scal_sb = small.tile([1, 3], F32, tag="scal_sb")
        nc.scalar.activation(scal_sb[:], scal_ps[:, 0:3], AF.Copy)
        nc.sync.dma_start(scal[:, :], scal_sb[:])

        counts_sb = small.tile([1, K], F32, tag="counts_sb")
        nc.scalar.activation(counts_sb[:], counts_ps[:], AF.Copy)
        nc.sync.dma_start(counts[:, :], counts_sb[:])

        nc.sync.dma_start(idxo[:, :], idx_all[:])


def build(T=T_FULL):
    if T in _NC_CACHE:
        return _NC_CACHE[T]
    nc = bacc.Bacc("TRN2", target_bir_lowering=False, debug=False,
                   num_devices=N_CORES)
    io = {}
    io["xt"] = nc.dram_tensor("xt", [D, T], F32, kind="ExternalInput").ap()
    io["et"] = nc.dram_tensor("et", [D, K], F32, kind="ExternalInput").ap()
    io["er"] = nc.dram_tensor("er", [K, D], F32, kind="ExternalInput").ap()
    io["bias3"] = nc.dram_tensor("bias3", [3, K], BF16, kind="ExternalInput").ap()
    io["cl"] = nc.dram_tensor("cl", [D, P], F32, kind="ExternalInput").ap()
    io["cr"] = nc.dram_tensor("cr", [D, K], F32, kind="ExternalInput").ap()
    io["q"] = nc.dram_tensor("q", [T, D], F32, kind="ExternalOutput").ap()
    io["idxo"] = nc.dram_tensor("idxo", [P, T // P], U16, kind="ExternalOutput").ap()
    io["counts"] = nc.dram_tensor("counts", [1, K], F32, kind="ExternalOutput").ap()
    io["scal"] = nc.dram_tensor("scal", [1, 3], F32, kind="ExternalOutput").ap()

    with tile.TileContext(nc) as tc:
        _emit(nc, tc, io, T)
    nc.compile()
    _NC_CACHE[T] = nc
    return nc


def _bf16_split3(b):
    bf = ml_dtypes.bfloat16
    hi = b.astype(bf)
    r1 = (b - hi.astype(np.float32)).astype(np.float32)
    mid = r1.astype(bf)
    lo = (r1 - mid.astype(np.float32)).astype(bf)
    return np.stack([hi, mid, lo.astype(bf)], axis=0)


def make_in_maps(inputs, embed_weight, T=T_FULL, n_cores=N_CORES):
    flat = np.ascontiguousarray(inputs.reshape(-1, D).astype(np.float32, copy=False))
    E = np.ascontiguousarray(embed_weight.astype(np.float32, copy=False))

    et = np.ascontiguousarray(E.T)
    e2 = (E.astype(np.float64) ** 2).sum(axis=1)
    bias3 = _bf16_split3((-0.5 * e2).astype(np.float32))

    norm = np.maximum(np.sqrt(e2), 1e-12)
    cbn = (E.astype(np.float64) / norm[:, None]).astype(np.float32)
    cr = np.ascontiguousarray(cbn.T)

    in_maps = []
    for c in range(n_cores):
        sh = flat[c * T:(c + 1) * T]
        in_maps.append({
            "xt": np.ascontiguousarray(sh.T),
            "et": et,
            "er": E,
            "bias3": bias3,
            "cl": np.ascontiguousarray(cr[:, c * P:(c + 1) * P]),
            "cr": cr,
        })
    aux = {"e2": e2, "cbn": cbn}
    return in_maps, aux


def combine(results, aux, in_shape, T=T_FULL, n_cores=N_CORES):
    N = T * n_cores
    q = np.concatenate([r["q"] for r in results], axis=0)
    quantized = q.reshape(in_shape)

    idx = np.concatenate(
        [r["idxo"].T.reshape(-1) for r in results]).astype(np.int32)

    counts = np.sum([r["counts"][0].astype(np.float64) for r in results], axis=0)
    scal = np.sum([r["scal"][0].astype(np.float64) for r in results], axis=0)
    smax_sum, xsq_sum, div_sum = scal[0], scal[1], scal[2]

    commitment = (xsq_sum - 2.0 * smax_sum) / (N * D)

    avg = counts / N
    u = 1.0 / K
    entropy = np.sum(u * (np.log(u) - np.log(avg + 1e-10)))

    cbn64 = aux["cbn"].astype(np.float64)
    diag = (cbn64 ** 2).sum(axis=1)          # sim_kk, ~1.0
    diag_sum = np.sum(diag ** 2)
    diversity = (div_sum - diag_sum + K * 0.01) / (K * K)

    total = 0.25 * commitment + 0.5 * diversity + 0.5 * entropy
    return quantized, np.float32(total), idx


def run(inputs, embed_weight, **kw):
    in_shape = inputs.shape
    nc = build(T_FULL)
    in_maps, aux = make_in_maps(np.asarray(inputs), np.asarray(embed_weight))
    res = bass_utils.run_bass_kernel_spmd(nc, in_maps,
                                          core_ids=list(range(N_CORES)), **kw)
    return combine(res.results, aux, in_shape), res


def kernel(inputs, embed_weight):
    out, _ = run(inputs, embed_weight)
    return out


# revision 36
# speedup vs baseline: 1.1184x; 1.1184x over previous
"""EnhancedVectorQuantizer forward on 8 Trainium2 NeuronCores.

Data-parallel over the flattened-token axis (65536 tokens -> 8 x 8192).
Codebook [1024, 256] replicated. Per core the Bass/Tile kernel computes:
  - scores s[t,k] = x_t . e_k - |e_k|^2/2 via a bf16 hi/lo split matmul
    (x1e1+x1e2+x2e1, fp32 PSUM accumulation; scheme error ~2e-5, below the
    fp32 reference's own rounding noise on the argmin; -e2/2 folded into the
    PSUM accumulation as a bf16x3-split bias row so no extra DVE pass)
  - argmax_k s = argmin_k |x_t - e_k|^2 (DVE max8 + max_index)
  - quantized rows via indirect-DMA gather from the DRAM codebook
  - per-code assignment counts (POOL one-hot, bf16 accumulate on DVE, one
    PE ones-matmul cross-partition reduce at the end)
  - partial sums for the losses:
      commitment: sum|x|^2 - 2*sum s_max   (algebraic identity, exact)
      diversity:  this core's 128-row slab of sum max(|cbn cbn^T|, 0.1)^2
The host only concatenates shards and does the O(K) scalar combine.
"""

import numpy as np
import ml_dtypes

import concourse.bacc as bacc
import concourse.bass as bass
import concourse.mybir as mybir
import concourse.tile as tile
from concourse import bass_utils
from concourse.bass import IndirectOffsetOnAxis

N_CORES = 8
D = 256
K = 1024
P = 128
T_FULL = 8192  # tokens per core

F32 = mybir.dt.float32
BF16 = mybir.dt.bfloat16
U16 = mybir.dt.uint16
U32 = mybir.dt.uint32

A = mybir.AluOpType
AF = mybir.ActivationFunctionType

_NC_CACHE = {}


def _emit_div(nc, sb, cl0, cl1, cr0, cr1, divacc, ps_d):
    # diversity: this core's 128-row slab of sim = cbn @ cbn.T
    for c in range(2):
        cs = slice(c * 512, (c + 1) * 512)
        d_ps = ps_d.tile([P, 512], F32, tag="d_ps")
        nc.tensor.matmul(d_ps[:], cl0[:], cr0[:, cs], start=True, stop=False)
        nc.tensor.matmul(d_ps[:], cl1[:], cr1[:, cs], start=False, stop=True)
        # max(|s|, 0.1)^2 == max(s^2, 0.01)
        divsq = sb.tile([P, 512], F32, tag="divsq")
        nc.scalar.activation(divsq[:], d_ps[:], AF.Square)
        divtmp = sb.tile([P, 512], F32, tag="divtmp")
        nc.vector.tensor_scalar(divtmp[:], divsq[:], 0.01, None, A.max,
                                A.add, accum_out=divacc[:, c:c + 1])


def _emit(nc, tc, io, T):
    NT = T // P           # token tiles
    QB = 4                # tiles per quantized-output DMA slab
    XCH = max(1, NT // 8)  # xt column-chunk size in tiles

    xt1, xt2, et1, et2, er, bias3, cl, cr = (
        io[k] for k in ("xt1", "xt2", "et1", "et2", "er", "bias3", "cl", "cr"))
    q, idxo, counts, scal = (io[k] for k in ("q", "idxo", "counts", "scal"))

    import contextlib
    ctx = contextlib.ExitStack()
    with ctx:
        const = ctx.enter_context(tc.tile_pool(name="const", bufs=1))
        sb = ctx.enter_context(tc.tile_pool(name="sb", bufs=4))
        ohp = ctx.enter_context(tc.tile_pool(name="ohp", bufs=4))
        small = ctx.enter_context(tc.tile_pool(name="small", bufs=2))
        qpool = ctx.enter_context(tc.tile_pool(name="qpool", bufs=2))
        ps_s = ctx.enter_context(tc.tile_pool(name="ps_s", bufs=2, space="PSUM"))
        ps_c = ctx.enter_context(tc.tile_pool(name="ps_c", bufs=1, space="PSUM"))
        ps_d = ctx.enter_context(tc.tile_pool(name="ps_d", bufs=1, space="PSUM"))
        ps_x = ctx.enter_context(tc.tile_pool(name="ps_x", bufs=1, space="PSUM"))

        # --- constants / whole-tensor loads ---
        # bf16 hi/lo codebook splits, per 128-row contraction block
        e1b = []
        e2b = []
        for b in range(2):
            t1 = const.tile([P, K], BF16, tag=f"e1b{b}")
            t2 = const.tile([P, K], BF16, tag=f"e2b{b}")
            nc.sync.dma_start(t1[:], et1[b * P:(b + 1) * P, :])
            nc.sync.dma_start(t2[:], et2[b * P:(b + 1) * P, :])
            e1b.append(t1)
            e2b.append(t2)

        bias3_sb = const.tile([3, K], BF16, tag="bias3")
        nc.sync.dma_start(bias3_sb[:], bias3[:, :])
        ones3 = const.tile([3, P], BF16, tag="ones3")
        nc.vector.memset(ones3[:], 1.0)
        ones128 = const.tile([P, 1], BF16, tag="ones128")
        nc.vector.memset(ones128[:], 1.0)
        ones1f = const.tile([P, 1], F32, tag="ones1f")
        nc.vector.memset(ones1f[:], 1.0)


        # xt hi/lo in column chunks so early tiles don't wait on the full load
        n_xch = NT // XCH
        xt_chunks = []   # (x1_blk0, x1_blk1, x2_blk0, x2_blk1) per chunk
        for cc in range(n_xch):
            s0, s1 = cc * XCH * P, (cc + 1) * XCH * P
            tiles = []
            for src, nmtag in ((xt1, "x1"), (xt2, "x2")):
                for b in range(2):
                    t = const.tile([P, XCH * P], BF16, tag=f"{nmtag}b{b}_{cc}")
                    nc.sync.dma_start(t[:], src[b * P:(b + 1) * P, s0:s1])
                    tiles.append(t)
            xt_chunks.append((tiles[0], tiles[1], tiles[2], tiles[3]))

        # diversity inputs: load early so the tail doesn't wait on DMA
        cl0 = const.tile([P, P], F32, tag="cl0")
        cl1 = const.tile([P, P], F32, tag="cl1")
        nc.sync.dma_start(cl0[:], cl[0:P, :])
        nc.sync.dma_start(cl1[:], cl[P:D, :])
        cr0 = const.tile([P, K], F32, tag="cr0")
        cr1 = const.tile([P, K], F32, tag="cr1")
        nc.sync.dma_start(cr0[:], cr[0:P, :])
        nc.sync.dma_start(cr1[:], cr[P:D, :])

        # accumulators
        idx_all = const.tile([P, NT], U16, tag="idx_all")
        max8_all = const.tile([P, 8 * NT], F32, tag="max8_all")
        idx8_all = const.tile([P, 8 * NT], U32, tag="idx8_all")
        xsq_all = const.tile([P, 2 * NT], F32, tag="xsq_all")
        counts_acc = const.tile([P, K], BF16, tag="counts_acc")
        nc.vector.memset(counts_acc[:], 0.0)
        oh_pend = []
        divacc = const.tile([P, 2], F32, tag="divacc")

        # --- main loop over token tiles ---
        for j in range(NT):
            cc, jj = divmod(j, XCH)
            js = slice(jj * P, (jj + 1) * P)
            x1b0 = xt_chunks[cc][0][:, js]
            x1b1 = xt_chunks[cc][1][:, js]
            x2b0 = xt_chunks[cc][2][:, js]
            x2b1 = xt_chunks[cc][3][:, js]

            s_ps = ps_s.tile([P, K], F32, tag="s_ps")
            for c in range(2):
                cs = slice(c * 512, (c + 1) * 512)
                # s = x1.e1 + x1.e2 + x2.e1 per contraction block (bf16 split;
                # dropped x2.e2 term is ~1e-5, below fp32 matmul noise)
                nc.tensor.matmul(s_ps[:, cs], x1b0, e1b[0][:, cs], start=True, stop=False)
                nc.tensor.matmul(s_ps[:, cs], x1b0, e2b[0][:, cs], start=False, stop=False)
                nc.tensor.matmul(s_ps[:, cs], x2b0, e1b[0][:, cs], start=False, stop=False)
                nc.tensor.matmul(s_ps[:, cs], x1b1, e1b[1][:, cs], start=False, stop=False)
                nc.tensor.matmul(s_ps[:, cs], x1b1, e2b[1][:, cs], start=False, stop=False)
                nc.tensor.matmul(s_ps[:, cs], x2b1, e1b[1][:, cs], start=False, stop=False)
                nc.tensor.matmul(s_ps[:, cs], ones3[:], bias3_sb[:, cs],
                                 start=False, stop=True)

            s_sb = sb.tile([P, K], F32, tag="s_sb")
            nc.scalar.activation(s_sb[:], s_ps[:], AF.Copy)

            max8 = max8_all[:, 8 * j:8 * (j + 1)]
            nc.vector.max(max8, s_sb[:])
            idx8 = idx8_all[:, 8 * j:8 * (j + 1)]
            nc.vector.max_index(idx8, max8, s_sb[:])

            # one-hot: score equals the per-token max (f32-exact tie prob ~0)
            oh = ohp.tile([P, K], BF16, tag="oh")
            nc.vector.tensor_scalar(oh[:], s_sb[:], max8_all[:, 8 * j:8 * j + 1],
                                    None, A.is_ge)
            # accumulate the PREVIOUS tile's one-hot: keeps the in-order DVE
            # stream from stalling on this tile's POOL op
            oh_pend.append(oh)
            if len(oh_pend) > 1:
                nc.vector.tensor_tensor(counts_acc[:], counts_acc[:],
                                        oh_pend.pop(0)[:], op=A.add)

            dummy = small.tile([P, P], F32, tag="dummy")
            nc.scalar.activation(dummy[:], x1b0, AF.Square,
                                 accum_out=xsq_all[:, 2 * j:2 * j + 1])
            dummy2 = small.tile([P, P], F32, tag="dummy")
            nc.scalar.activation(dummy2[:], x1b1, AF.Square,
                                 accum_out=xsq_all[:, 2 * j + 1:2 * j + 2])

            # gather quantized rows for this tile
            a = j % QB
            if a == 0:
                q_sb = qpool.tile([P, QB * D], F32, tag="q_sb")
            nc.gpsimd.indirect_dma_start(
                out=q_sb[:, a * D:(a + 1) * D],
                out_offset=None,
                in_=er[:, :],
                in_offset=IndirectOffsetOnAxis(
                    ap=idx8_all[:, 8 * j:8 * j + 1], axis=0),
            )
            if a == QB - 1:
                j0 = j - (QB - 1)
                slab = q[j0 * P:(j0 + QB) * P, :].rearrange(
                    "(a p) d -> p a d", p=P)
                nc.sync.dma_start(slab, q_sb[:].rearrange("p (a d) -> p a d", d=D))

        while oh_pend:
            nc.vector.tensor_tensor(counts_acc[:], counts_acc[:],
                                    oh_pend.pop(0)[:], op=A.add)

        _emit_div(nc, sb, cl0, cl1, cr0, cr1, divacc, ps_d)

        # --- counts: cross-partition reduce of the bf16 accumulator ---
        counts_ps = ps_c.tile([1, K], F32)
        nc.tensor.matmul(counts_ps[:, 0:512], ones128[:], counts_acc[:, 0:512],
                         start=True, stop=True)
        nc.tensor.matmul(counts_ps[:, 512:1024], ones128[:], counts_acc[:, 512:1024],
                         start=True, stop=True)


        # --- final reductions -> scal = [sum s_max, sum |x|^2, div_part] ---
        nc.vector.tensor_copy(
            idx_all[:], idx8_all[:].rearrange("p (j e) -> p j e", e=8)[:, :, 0:1])
        smax_red = small.tile([P, 1], F32, tag="smax_red")
        nc.vector.tensor_reduce(
            smax_red[:], max8_all[:].rearrange("p (j e) -> p j e", e=8)[:, :, 0:1],
            axis=mybir.AxisListType.XY, op=A.add)
        xsq_red = small.tile([P, 1], F32, tag="xsq_red")
        nc.vector.reduce_sum(xsq_red[:], xsq_all[:], axis=mybir.AxisListType.X)
        div_red = small.tile([P, 1], F32, tag="div_red")
        nc.vector.reduce_sum(div_red[:], divacc[:], axis=mybir.AxisListType.X)

        scal_ps = ps_x.tile([1, 4], F32)
        nc.tensor.matmul(scal_ps[:, 0:1], smax_red[:], ones1f[:], start=True, stop=True)
        nc.tensor.matmul(scal_ps[:, 1:2], xsq_red[:], ones1f[:], start=True, stop=True)
        nc.tensor.matmul(scal_ps[:, 2:3], div_red[:], ones1f[:], start=True, stop=True)

        scal_sb = small.tile([1, 3], F32, tag="scal_sb")
        nc.scalar.activation(scal_sb[:], scal_ps[:, 0:3], AF.Copy)
        nc.sync.dma_start(scal[:, :], scal_sb[:])

        counts_sb = small.tile([1, K], F32, tag="counts_sb")
        nc.scalar.activation(counts_sb[:], counts_ps[:], AF.Copy)
        nc.sync.dma_start(counts[:, :], counts_sb[:])

        nc.sync.dma_start(idxo[:, :], idx_all[:])


def build(T=T_FULL):
    if T in _NC_CACHE:
        return _NC_CACHE[T]
    nc = bacc.Bacc("TRN2", target_bir_lowering=False, debug=False,
                   num_devices=N_CORES)
    io = {}
    io["xt1"] = nc.dram_tensor("xt1", [D, T], BF16, kind="ExternalInput").ap()
    io["xt2"] = nc.dram_tensor("xt2", [D, T], BF16, kind="ExternalInput").ap()
    io["et1"] = nc.dram_tensor("et1", [D, K], BF16, kind="ExternalInput").ap()
    io["et2"] = nc.dram_tensor("et2", [D, K], BF16, kind="ExternalInput").ap()
    io["er"] = nc.dram_tensor("er", [K, D], F32, kind="ExternalInput").ap()
    io["bias3"] = nc.dram_tensor("bias3", [3, K], BF16, kind="ExternalInput").ap()
    io["cl"] = nc.dram_tensor("cl", [D, P], F32, kind="ExternalInput").ap()
    io["cr"] = nc.dram_tensor("cr", [D, K], F32, kind="ExternalInput").ap()
    io["q"] = nc.dram_tensor("q", [T, D], F32, kind="ExternalOutput").ap()
    io["idxo"] = nc.dram_tensor("idxo", [P, T // P], U16, kind="ExternalOutput").ap()
    io["counts"] = nc.dram_tensor("counts", [1, K], F32, kind="ExternalOutput").ap()
    io["scal"] = nc.dram_tensor("scal", [1, 3], F32, kind="ExternalOutput").ap()

    with tile.TileContext(nc) as tc:
        _emit(nc, tc, io, T)
    nc.compile()
    _NC_CACHE[T] = nc
    return nc


def _bf16_split2(v):
    bf = ml_dtypes.bfloat16
    hi = v.astype(bf)
    lo = (v - hi.astype(np.float32)).astype(bf)
    return hi, lo


def _bf16_split3(b):
    bf = ml_dtypes.bfloat16
    hi = b.astype(bf)
    r1 = (b - hi.astype(np.float32)).astype(np.float32)
    mid = r1.astype(bf)
    lo = (r1 - mid.astype(np.float32)).astype(bf)
    return np.stack([hi, mid, lo.astype(bf)], axis=0)


def make_in_maps(inputs, embed_weight, T=T_FULL, n_cores=N_CORES):
    flat = np.ascontiguousarray(inputs.reshape(-1, D).astype(np.float32, copy=False))
    E = np.ascontiguousarray(embed_weight.astype(np.float32, copy=False))

    etf = np.ascontiguousarray(E.T)
    et1, et2 = _bf16_split2(etf)
    e2 = (E.astype(np.float64) ** 2).sum(axis=1)
    bias3 = _bf16_split3((-0.5 * e2).astype(np.float32))

    norm = np.maximum(np.sqrt(e2), 1e-12)
    cbn = (E.astype(np.float64) / norm[:, None]).astype(np.float32)
    cr = np.ascontiguousarray(cbn.T)

    in_maps = []
    for c in range(n_cores):
        sh = np.ascontiguousarray(flat[c * T:(c + 1) * T].T)
        x1, x2 = _bf16_split2(sh)
        in_maps.append({
            "xt1": x1,
            "xt2": x2,
            "et1": et1,
            "et2": et2,
            "er": E,
            "bias3": bias3,
            "cl": np.ascontiguousarray(cr[:, c * P:(c + 1) * P]),
            "cr": cr,
        })
    aux = {"e2": e2, "cbn": cbn}
    return in_maps, aux


def combine(results, aux, in_shape, T=T_FULL, n_cores=N_CORES):
    N = T * n_cores
    q = np.concatenate([r["q"] for r in results], axis=0)
    quantized = q.reshape(in_shape)

    idx = np.concatenate(
        [r["idxo"].T.reshape(-1) for r in results]).astype(np.int32)

    counts = np.sum([r["counts"][0].astype(np.float64) for r in results], axis=0)
    scal = np.sum([r["scal"][0].astype(np.float64) for r in results], axis=0)
    smax_sum, xsq_sum, div_sum = scal[0], scal[1], scal[2]

    commitment = (xsq_sum - 2.0 * smax_sum) / (N * D)

    avg = counts / N
    u = 1.0 / K
    entropy = np.sum(u * (np.log(u) - np.log(avg + 1e-10)))

    cbn64 = aux["cbn"].astype(np.float64)
    diag = (cbn64 ** 2).sum(axis=1)          # sim_kk, ~1.0
    diag_sum = np.sum(diag ** 2)
    diversity = (div_sum - diag_sum + K * 0.01) / (K * K)

    total = 0.25 * commitment + 0.5 * diversity + 0.5 * entropy
    return quantized, np.float32(total), idx


def run(inputs, embed_weight, **kw):
    in_shape = inputs.shape
    nc = build(T_FULL)
    in_maps, aux = make_in_maps(np.asarray(inputs), np.asarray(embed_weight))
    res = bass_utils.run_bass_kernel_spmd(nc, in_maps,
                                          core_ids=list(range(N_CORES)), **kw)
    return combine(res.results, aux, in_shape), res


def kernel(inputs, embed_weight):
    out, _ = run(inputs, embed_weight)
    return out
